# revision 5
# baseline (speedup 1.0000x reference)
"""Trainium2 Bass kernel for nn_Net_73710228734901.

The network's post-gather graph (concat -> Conv3d -> spatial mean -> Linear)
is entirely linear in the gathered pixels, and the gathers / avg-pool /
1x1-conv are linear in the inputs.  Since the output is only [B, 1], the
whole model collapses to

    out[b] = lin_b + <W1, x1crop[b]> + <W2, x2crop[b]> + <W4, sharecrop[b]>
                   + <W3, x3[b]>

with fixed per-element weight tensors computed (cheaply, on host) from
c_w / conv3d_w / lin_w / idx_h / idx_w.  Only the 7x7 per-channel crop
windows of x1/x2/share carry nonzero weight, so the host packs just those
49 of 196 positions per channel (pure indexing); x3's weights are dense
(the 1x1 conv mixes all channels), so all of x3 streams.

Device kernel (per core, channel-sharded 8 ways): the whole reduction runs
on the TensorEngine as a chain of [128,1]^T @ [128,64] matvec matmuls
accumulating in PSUM.  Chunk k holds 128 consecutive elements of the
core's (crops ++ x3) stream across partitions, for all 64 batches; lhsT
is the matching fp16 weight column.  Products are exact (fp16 in, fp32
accumulate).  The 147 crop chunks additionally run a second matmul with
the fp16 *residual* weight column into a second PSUM bank, recovering
~fp32 weight precision for the crops at no extra DMA cost.  ~35 ns per
chunk on the PE; DVE/ACT stay idle.

Per-core HBM traffic = 18.5 MB of fp16 activations, the memory roofline
for this problem; the PE chain (~45 us) hides entirely under the DMA
stream (~53 us).
"""

import numpy as np

import concourse.bacc as bacc
import concourse.mybir as mybir
from concourse.bass_utils import run_bass_kernel_spmd
from concourse.tile import TileContext

NCORES = 8
NB = 64            # full batch, all on every core (channel sharding)
NCROP = 147        # 3 * 49 crop elems per (partition, batch)
NCH3 = 980         # x3 elems per partition: 160 ch * 784 pos / 128
NCHUNK = NCROP + NCH3   # 1127 PE chunks of 128 elements
CPT = 140          # PE chunks per DMA tile
PEBUFS = 4         # PE x-tile buffer depth
HILO = True        # double-fp16 weights for the crop chunks
W_SCALE = 1024.0   # weights pre-scaled so fp16 values avoid subnormals
LO_SCALE = 2048.0  # extra scale on the residual (lo) weight columns

_F32 = mybir.dt.float32
_F16 = mybir.dt.float16


def _build_fold(c_w, conv3d_w, lin_w, lin_b, idx_h, idx_w):
    """Collapse conv3d+mean+linear into per-element weights (float64 host).

    Returns A: [1024, 14, 14] quadrant weights (applied to the gathered
    crops directly) and Ws3: [1280, 784] dense weights for raw x3.
    """
    c_w = c_w.astype(np.float64)
    conv3d_w = conv3d_w.astype(np.float64)
    lin_w = lin_w.astype(np.float64)

    # W2[c = i*64+dd, kh, kw] = sum_{o,d,kd: 3d-4+kd=dd} lin_w[o*24+d]
    #                           * conv3d_w[o,i,kd,kh,kw]
    W2 = np.zeros((1024, 3, 3), np.float64)
    o_idx = np.arange(32) * 24
    i_idx = np.arange(16) * 64
    for d in range(24):
        for kd in range(3):
            dd = 3 * d - 4 + kd
            if 0 <= dd < 64:
                W2[i_idx + dd] += np.einsum(
                    'o,oikl->ikl', lin_w[o_idx + d, 0], conv3d_w[:, :, kd])

    # Mean over the 14x14 conv output folds each (kh,kw) tap into a
    # border mask.
    M = np.zeros((3, 3, 14, 14), np.float64)
    rng = {0: (0, 13), 1: (0, 14), 2: (1, 14)}
    for kh in range(3):
        for kw in range(3):
            r0, r1 = rng[kh]
            c0, c1 = rng[kw]
            M[kh, kw, r0:r1, c0:c1] = 1.0
    A = np.einsum('ckl,klrs->crs', W2, M) / 196.0   # [1024, 14, 14]

    # x3 path: scatter quadrant 3's 7x7 weights to the pooled 14x14 grid
    # at the per-channel crop offset, pull back through the 1x1 conv and
    # the transposed avg_pool2d(5, stride 2, pad 2).
    Aq3 = A[:, 0:7, 7:14]
    Ws3c = np.zeros((1024, 14, 14), np.float64)
    ci = np.arange(1024)[:, None, None]
    ri = (idx_h[2][:, None] + np.arange(7))[:, :, None]
    wi = (idx_w[2][:, None] + np.arange(7))[:, None, :]
    Ws3c[ci, ri, wi] = Aq3
    Wpool = np.einsum('oc,ohw->chw', c_w, Ws3c)     # [1280, 14, 14]
    Ws3 = np.zeros((1280, 28, 28), np.float64)
    for dh in range(-2, 3):
        for dw in range(-2, 3):
            hs = [h for h in range(14) if 0 <= 2 * h + dh < 28]
            ws = [w for w in range(14) if 0 <= 2 * w + dw < 28]
            H = [2 * h + dh for h in hs]
            W_ = [2 * w + dw for w in ws]
            Ws3[:, np.ix_(H, W_)[0], np.ix_(H, W_)[1]] += \
                Wpool[:, np.ix_(hs, ws)[0], np.ix_(hs, ws)[1]] / 25.0

    return A, Ws3.reshape(1280, 784)


def _crop(x, ih, iw):
    """Gather per-channel 7x7 crops: [B,1024,14,14] -> [B,1024,49]."""
    c = np.arange(x.shape[1])[None, :, None, None]
    r = (ih[:, None] + np.arange(7))[None, :, :, None]
    w = (iw[:, None] + np.arange(7))[None, :, None, :]
    return x[:, c, r, w].reshape(x.shape[0], x.shape[1], 49)


def _build_bass(cpt=CPT, pebufs=PEBUFS, hilo=HILO):
    nlo = NCROP if hilo else 0
    ntiles = (NCHUNK + cpt - 1) // cpt

    nc = bacc.Bacc("TRN2")
    xpe = nc.dram_tensor("xpe", [128, NCHUNK, NB], _F16, kind="ExternalInput")
    wpe = nc.dram_tensor("wpe", [128, NCHUNK + nlo], _F16,
                         kind="ExternalInput")
    linb = nc.dram_tensor("linb", [1, 1], _F32, kind="ExternalInput")
    out = nc.dram_tensor("out", [1, NB], _F32, kind="ExternalOutput")

    with TileContext(nc) as tc:
        with (
            tc.tile_pool(name="cpool", bufs=1) as cpool,
            tc.tile_pool(name="pepool", bufs=pebufs) as pepool,
            tc.tile_pool(name="apool", bufs=1) as apool,
            tc.tile_pool(name="ppool", bufs=2, space="PSUM") as ppool,
        ):
            # Weights + bias issue from the scalar queue so the data
            # stream owns the sync queue from t=0.
            wp = cpool.tile([128, NCHUNK + nlo], _F16)
            nc.scalar.dma_start(out=wp[:], in_=wpe[:, :])
            lb = cpool.tile([1, 1], _F32)
            nc.scalar.dma_start(out=lb[:], in_=linb[:, :])

            ps_hi = ppool.tile([1, NB], _F32)
            ps_lo = ppool.tile([1, NB], _F32)
            for t in range(ntiles):
                k0 = t * cpt
                k1 = min(k0 + cpt, NCHUNK)
                pt = pepool.tile([128, (k1 - k0) * NB], _F16, tag="pt")
                nc.sync.dma_start(out=pt[:], in_=xpe[:, k0:k1, :])
                for k in range(k0, k1):
                    rhs = pt[:, (k - k0) * NB:(k - k0 + 1) * NB]
                    nc.tensor.matmul(
                        ps_hi[:], lhsT=wp[:, k:k + 1], rhs=rhs,
                        start=(k == 0), stop=(k == NCHUNK - 1))
                    if k < nlo:
                        nc.tensor.matmul(
                            ps_lo[:], lhsT=wp[:, NCHUNK + k:NCHUNK + k + 1],
                            rhs=rhs, start=(k == 0), stop=(k == nlo - 1))

            # Combine the two PSUM registers, undo the weight pre-scales,
            # add lin_b.
            res = apool.tile([1, NB], _F32)
            if nlo:
                half = apool.tile([1, NB], _F32)
                nc.vector.tensor_scalar(
                    half[:], ps_lo[:], 1.0 / (W_SCALE * LO_SCALE), lb[:],
                    mybir.AluOpType.mult, mybir.AluOpType.add)
                nc.vector.scalar_tensor_tensor(
                    out=res[:], in0=ps_hi[:], scalar=1.0 / W_SCALE,
                    in1=half[:],
                    op0=mybir.AluOpType.mult, op1=mybir.AluOpType.add)
            else:
                nc.vector.tensor_scalar(
                    res[:], ps_hi[:], 1.0 / W_SCALE, lb[:],
                    mybir.AluOpType.mult, mybir.AluOpType.add)
            nc.sync.dma_start(out=out[:, :], in_=res[:])
    nc.finalize()
    return nc


def _shard_inputs(x1, x2, x3, share_feature, A, Ws3, lin_b, idx_h, idx_w,
                  hilo=HILO):
    """Gather crops, pack per-core PE-layout tensors (fp16)."""
    nlo = NCROP if hilo else 0

    xc1 = _crop(x1, idx_h[0], idx_w[0])              # [64, 1024, 49]
    xc2 = _crop(x2, idx_h[1], idx_w[1])
    xcs = _crop(share_feature, idx_h[3], idx_w[3])
    Aq1 = A[:, 0:7, 0:7].reshape(1024, 49)
    Aq2 = A[:, 7:14, 0:7].reshape(1024, 49)
    Aq4 = A[:, 7:14, 7:14].reshape(1024, 49)
    x3f = np.asarray(x3, np.float32).reshape(NB, 1280 * 784)
    w3f = Ws3.reshape(1280 * 784)

    in_maps = []
    for m in range(NCORES):
        cs = slice(m * 128, (m + 1) * 128)
        e0, e1 = m * 160 * 784, (m + 1) * 160 * 784

        # Per-core flat element stream: crops (ch-major) ++ x3 slice.
        cropx = np.concatenate(
            [xc1[:, cs], xc2[:, cs], xcs[:, cs]], axis=2)    # [64, 128, 147]
        xall = np.concatenate(
            [cropx.reshape(NB, 128 * NCROP), x3f[:, e0:e1]], axis=1)
        # chunks: [64, 1127, 128] -> [128, 1127, 64]
        xpe = np.ascontiguousarray(
            xall.reshape(NB, NCHUNK, 128).transpose(2, 1, 0),
            dtype=np.float16)

        cropw = np.concatenate(
            [Aq1[cs], Aq2[cs], Aq4[cs]], axis=1)             # [128, 147]
        wall = np.concatenate(
            [cropw.reshape(128 * NCROP), w3f[e0:e1]]) * W_SCALE
        whi = wall.reshape(NCHUNK, 128).T.astype(np.float16)  # [128, 1127]
        if nlo:
            wlo = ((wall[:128 * NCROP]
                    - whi.T.reshape(NCHUNK * 128)[:128 * NCROP]
                        .astype(np.float64))
                   * LO_SCALE).reshape(nlo, 128).T.astype(np.float16)
            wpe = np.ascontiguousarray(
                np.concatenate([whi, wlo], axis=1), dtype=np.float16)
        else:
            wpe = np.ascontiguousarray(whi, dtype=np.float16)

        linb = np.array([[lin_b[0] if m == 0 else 0.0]], np.float32)
        in_maps.append({'xpe': xpe, 'wpe': wpe, 'linb': linb})
    return in_maps


def _prepare(inputs):
    """Fold weights + shard; returns (nc, in_maps)."""
    A, Ws3 = _build_fold(
        np.asarray(inputs['c_w']), np.asarray(inputs['conv3d_w']),
        np.asarray(inputs['lin_w']), np.asarray(inputs['lin_b']),
        np.asarray(inputs['idx_h']), np.asarray(inputs['idx_w']))
    in_maps = _shard_inputs(
        np.asarray(inputs['x1']), np.asarray(inputs['x2']),
        np.asarray(inputs['x3']), np.asarray(inputs['share_feature']),
        A, Ws3, np.asarray(inputs['lin_b']),
        np.asarray(inputs['idx_h']), np.asarray(inputs['idx_w']))
    nc = _build_bass()
    return nc, in_maps


def _ensure_ntff_hook():
    """Make `trace=True` (e.g. BASS_TRACE=1) work under axon even when the
    image's antenv package lacks axon_hooks: register an equivalent module
    backed by the ctypes NTFF hook from trn_agent_boot."""
    import sys
    import types
    try:
        import antenv.axon_hooks  # noqa: F401
        return
    except Exception:
        pass
    try:
        from trn_agent_boot import trn_boot
        hook = trn_boot._ntff_profile_via_ctypes('/opt/axon/libaxon_pjrt.so')
        mod = types.ModuleType('antenv.axon_hooks')
        mod.get_axon_ntff_profile_hook = lambda: hook
        mod.set_axon_ntff_profile_hook = lambda h: None
        sys.modules['antenv.axon_hooks'] = mod
    except Exception:
        pass


def kernel(x1, x2, x3, share_feature, c_w, conv3d_w, lin_w, lin_b,
           idx_h, idx_w):
    _ensure_ntff_hook()
    nc, in_maps = _prepare({
        'x1': x1, 'x2': x2, 'x3': x3, 'share_feature': share_feature,
        'c_w': c_w, 'conv3d_w': conv3d_w, 'lin_w': lin_w, 'lin_b': lin_b,
        'idx_h': idx_h, 'idx_w': idx_w})
    res = run_bass_kernel_spmd(nc, in_maps, core_ids=list(range(NCORES)))
    parts = np.stack([r['out'][0] for r in res.results])      # [8, 64]
    return parts.sum(axis=0, dtype=np.float64).astype(np.float32).reshape(
        NB, 1)


# revision 8
# speedup vs baseline: 1.1038x; 1.1038x over previous
"""Trainium2 Bass kernel for nn_Net_73710228734901.

The network's post-gather graph (concat -> Conv3d -> spatial mean -> Linear)
is entirely linear in the gathered pixels, and the gathers / avg-pool /
1x1-conv are linear in the inputs.  Since the output is only [B, 1], the
whole model collapses to

    out[b] = lin_b + <W1, x1crop[b]> + <W2, x2crop[b]> + <W4, sharecrop[b]>
                   + <W3, x3[b]>

with fixed per-element weight tensors computed (cheaply, on host) from
c_w / conv3d_w / lin_w / idx_h / idx_w.  Only the 7x7 per-channel crop
windows of x1/x2/share carry nonzero weight, so the host packs just those
49 of 196 positions per channel (pure indexing); x3's weights are dense
(the 1x1 conv mixes all channels), so all of x3 streams.

Device kernel (per core, channel-sharded 8 ways): the whole reduction runs
on the TensorEngine as a chain of [128,1]^T @ [128,64] matvec matmuls
accumulating in PSUM.  Chunk k holds 128 consecutive elements of the
core's (crops ++ x3) stream across partitions, for all 64 batches; lhsT
is the matching fp16 weight column.  Products are exact (fp16 in, fp32
accumulate).  The 147 crop chunks additionally run a second matmul with
the fp16 *residual* weight column into a second PSUM bank, recovering
~fp32 weight precision for the crops at no extra DMA cost.  ~35 ns per
chunk on the PE; DVE/ACT stay idle.

Per-core HBM traffic = 18.5 MB of fp16 activations, the memory roofline
for this problem; the PE chain (~45 us) hides entirely under the DMA
stream (~53 us).
"""

import numpy as np

import concourse.bacc as bacc
import concourse.mybir as mybir
from concourse.bass_utils import run_bass_kernel_spmd
from concourse.tile import TileContext

NCORES = 8
NB = 64            # full batch, all on every core (channel sharding)
NCROP = 147        # 3 * 49 crop elems per (partition, batch)
NCH3 = 980         # x3 elems per partition: 160 ch * 784 pos / 128
NCHUNK = NCROP + NCH3   # 1127 PE chunks of 128 elements
CPT = 70           # PE chunks per DMA tile
PEBUFS = 4         # PE x-tile buffer depth
HILO = True        # double-fp16 weights for the crop chunks
W_SCALE = 1024.0   # weights pre-scaled so fp16 values avoid subnormals
LO_SCALE = 2048.0  # extra scale on the residual (lo) weight columns

_F32 = mybir.dt.float32
_F16 = mybir.dt.float16


def _build_fold(c_w, conv3d_w, lin_w, lin_b, idx_h, idx_w):
    """Collapse conv3d+mean+linear into per-element weights (float64 host).

    Returns A: [1024, 14, 14] quadrant weights (applied to the gathered
    crops directly) and Ws3: [1280, 784] dense weights for raw x3.
    """
    c_w = c_w.astype(np.float64)
    conv3d_w = conv3d_w.astype(np.float64)
    lin_w = lin_w.astype(np.float64)

    # W2[c = i*64+dd, kh, kw] = sum_{o,d,kd: 3d-4+kd=dd} lin_w[o*24+d]
    #                           * conv3d_w[o,i,kd,kh,kw]
    W2 = np.zeros((1024, 3, 3), np.float64)
    o_idx = np.arange(32) * 24
    i_idx = np.arange(16) * 64
    for d in range(24):
        for kd in range(3):
            dd = 3 * d - 4 + kd
            if 0 <= dd < 64:
                W2[i_idx + dd] += np.einsum(
                    'o,oikl->ikl', lin_w[o_idx + d, 0], conv3d_w[:, :, kd])

    # Mean over the 14x14 conv output folds each (kh,kw) tap into a
    # border mask.
    M = np.zeros((3, 3, 14, 14), np.float64)
    rng = {0: (0, 13), 1: (0, 14), 2: (1, 14)}
    for kh in range(3):
        for kw in range(3):
            r0, r1 = rng[kh]
            c0, c1 = rng[kw]
            M[kh, kw, r0:r1, c0:c1] = 1.0
    A = np.einsum('ckl,klrs->crs', W2, M) / 196.0   # [1024, 14, 14]

    # x3 path: scatter quadrant 3's 7x7 weights to the pooled 14x14 grid
    # at the per-channel crop offset, pull back through the 1x1 conv and
    # the transposed avg_pool2d(5, stride 2, pad 2).
    Aq3 = A[:, 0:7, 7:14]
    Ws3c = np.zeros((1024, 14, 14), np.float64)
    ci = np.arange(1024)[:, None, None]
    ri = (idx_h[2][:, None] + np.arange(7))[:, :, None]
    wi = (idx_w[2][:, None] + np.arange(7))[:, None, :]
    Ws3c[ci, ri, wi] = Aq3
    Wpool = np.einsum('oc,ohw->chw', c_w, Ws3c)     # [1280, 14, 14]
    Ws3 = np.zeros((1280, 28, 28), np.float64)
    for dh in range(-2, 3):
        for dw in range(-2, 3):
            hs = [h for h in range(14) if 0 <= 2 * h + dh < 28]
            ws = [w for w in range(14) if 0 <= 2 * w + dw < 28]
            H = [2 * h + dh for h in hs]
            W_ = [2 * w + dw for w in ws]
            Ws3[:, np.ix_(H, W_)[0], np.ix_(H, W_)[1]] += \
                Wpool[:, np.ix_(hs, ws)[0], np.ix_(hs, ws)[1]] / 25.0

    return A, Ws3.reshape(1280, 784)


def _crop(x, ih, iw):
    """Gather per-channel 7x7 crops: [B,1024,14,14] -> [B,1024,49]."""
    c = np.arange(x.shape[1])[None, :, None, None]
    r = (ih[:, None] + np.arange(7))[None, :, :, None]
    w = (iw[:, None] + np.arange(7))[None, :, None, :]
    return x[:, c, r, w].reshape(x.shape[0], x.shape[1], 49)


def _tiles(cpt=CPT):
    """Chunk ranges per DMA transfer; runt tile last to shorten the tail."""
    ts = []
    k = 0
    while k < NCHUNK:
        k1 = min(k + cpt, NCHUNK)
        ts.append((k, k1))
        k = k1
    return ts


def _build_bass_raw(cpt=CPT, hilo=HILO):
    """Raw (non-Tile) build: whole input resident in SBUF, manual sems.

    Avoids the Tile framework's ~8 us prologue (pool/sem setup) and ~10 us
    epilogue (sem recycling ladder + multi-round exit barriers): one
    counting semaphore tracks the in-order data-DMA stream, the PE chain
    waits per tile, and the program ends right after the output DMA
    confirms.
    """
    nlo = NCROP if hilo else 0
    tiles = _tiles(cpt)

    nc = bacc.Bacc("TRN2")
    xpe = nc.dram_tensor("xpe", [128, NCHUNK, NB], _F16, kind="ExternalInput")
    wpe = nc.dram_tensor("wpe", [128, NCHUNK + nlo], _F16,
                         kind="ExternalInput")
    linb = nc.dram_tensor("linb", [1, 1], _F32, kind="ExternalInput")
    out = nc.dram_tensor("out", [1, NB], _F32, kind="ExternalOutput")

    xt = nc.alloc_sbuf_tensor("xt", [128, NCHUNK * NB], _F16)
    wp = nc.alloc_sbuf_tensor("wp", [128, NCHUNK + nlo], _F16)
    lbt = nc.alloc_sbuf_tensor("lbt", [1, 1], _F32)
    half = nc.alloc_sbuf_tensor("half", [1, NB], _F32)
    res = nc.alloc_sbuf_tensor("res", [1, NB], _F32)
    ps_hi = nc.alloc_psum_tensor("ps_hi", [1, NB], _F32)
    ps_lo = nc.alloc_psum_tensor("ps_lo", [1, NB], _F32)

    dsem = nc.alloc_semaphore("dsem")
    wsem = nc.alloc_semaphore("wsem")
    csem = nc.alloc_semaphore("csem")
    osem = nc.alloc_semaphore("osem")

    # Prologue: zero our sems (values persist across executions), then
    # fence so no DMA can complete before the clear.
    for s in (dsem, wsem, csem, osem):
        nc.gpsimd.sem_clear(s)
    nc.all_engine_barrier()

    # Weights + bias from the scalar queue; the data stream owns sync.
    nc.scalar.dma_start(wp[:, :], wpe[:, :]).then_inc(wsem, 16)
    nc.scalar.dma_start(lbt[:, :], linb[:, :]).then_inc(wsem, 16)
    for (k0, k1) in tiles:
        nc.sync.dma_start(
            xt[:, k0 * NB:k1 * NB], xpe[:, k0:k1, :]).then_inc(dsem, 16)

    # PE chain: per tile, wait for its (in-order) completion count.
    nc.tensor.wait_ge(wsem, 32)
    for t, (k0, k1) in enumerate(tiles):
        nc.tensor.wait_ge(dsem, 16 * (t + 1))
        for k in range(k0, k1):
            rhs = xt[:, k * NB:(k + 1) * NB]
            mm = nc.tensor.matmul(
                ps_hi[:, :], lhsT=wp[:, k:k + 1], rhs=rhs,
                start=(k == 0), stop=(k == NCHUNK - 1))
            if k == NCHUNK - 1:
                mm.then_inc(csem, 1)
            if k < nlo:
                mm2 = nc.tensor.matmul(
                    ps_lo[:, :], lhsT=wp[:, NCHUNK + k:NCHUNK + k + 1],
                    rhs=rhs, start=(k == 0), stop=(k == nlo - 1))
                if k == nlo - 1:
                    mm2.then_inc(csem, 1)

    # Combine on DVE, then the output DMA; hold the sync queue until the
    # write is confirmed so execution cannot retire early.
    nc.vector.wait_ge(csem, 2 if nlo else 1)
    if nlo:
        nc.vector.tensor_scalar(
            half[:, :], ps_lo[:, :], 1.0 / (W_SCALE * LO_SCALE), lbt[:, :],
            mybir.AluOpType.mult, mybir.AluOpType.add)
        nc.vector.scalar_tensor_tensor(
            out=res[:, :], in0=ps_hi[:, :], scalar=1.0 / W_SCALE,
            in1=half[:, :], op0=mybir.AluOpType.mult,
            op1=mybir.AluOpType.add).then_inc(csem, 1)
    else:
        nc.vector.tensor_scalar(
            res[:, :], ps_hi[:, :], 1.0 / W_SCALE, lbt[:, :],
            mybir.AluOpType.mult, mybir.AluOpType.add).then_inc(csem, 1)
    nc.sync.wait_ge(csem, 3 if nlo else 2)
    nc.sync.dma_start(out[:, :], res[:, :]).then_inc(osem, 16)
    nc.sync.wait_ge(osem, 16)
    nc.finalize()
    return nc


def _build_bass(cpt=CPT, pebufs=PEBUFS, hilo=HILO):
    nlo = NCROP if hilo else 0
    ntiles = (NCHUNK + cpt - 1) // cpt

    nc = bacc.Bacc("TRN2")
    xpe = nc.dram_tensor("xpe", [128, NCHUNK, NB], _F16, kind="ExternalInput")
    wpe = nc.dram_tensor("wpe", [128, NCHUNK + nlo], _F16,
                         kind="ExternalInput")
    linb = nc.dram_tensor("linb", [1, 1], _F32, kind="ExternalInput")
    out = nc.dram_tensor("out", [1, NB], _F32, kind="ExternalOutput")

    with TileContext(nc) as tc:
        with (
            tc.tile_pool(name="cpool", bufs=1) as cpool,
            tc.tile_pool(name="pepool", bufs=pebufs) as pepool,
            tc.tile_pool(name="apool", bufs=1) as apool,
            tc.tile_pool(name="ppool", bufs=2, space="PSUM") as ppool,
        ):
            # Weights + bias issue from the scalar queue so the data
            # stream owns the sync queue from t=0.
            wp = cpool.tile([128, NCHUNK + nlo], _F16)
            nc.scalar.dma_start(out=wp[:], in_=wpe[:, :])
            lb = cpool.tile([1, 1], _F32)
            nc.scalar.dma_start(out=lb[:], in_=linb[:, :])

            ps_hi = ppool.tile([1, NB], _F32)
            ps_lo = ppool.tile([1, NB], _F32)
            for t in range(ntiles):
                k0 = t * cpt
                k1 = min(k0 + cpt, NCHUNK)
                pt = pepool.tile([128, (k1 - k0) * NB], _F16, tag="pt")
                nc.sync.dma_start(out=pt[:], in_=xpe[:, k0:k1, :])
                for k in range(k0, k1):
                    rhs = pt[:, (k - k0) * NB:(k - k0 + 1) * NB]
                    nc.tensor.matmul(
                        ps_hi[:], lhsT=wp[:, k:k + 1], rhs=rhs,
                        start=(k == 0), stop=(k == NCHUNK - 1))
                    if k < nlo:
                        nc.tensor.matmul(
                            ps_lo[:], lhsT=wp[:, NCHUNK + k:NCHUNK + k + 1],
                            rhs=rhs, start=(k == 0), stop=(k == nlo - 1))

            # Combine the two PSUM registers, undo the weight pre-scales,
            # add lin_b.
            res = apool.tile([1, NB], _F32)
            if nlo:
                half = apool.tile([1, NB], _F32)
                nc.vector.tensor_scalar(
                    half[:], ps_lo[:], 1.0 / (W_SCALE * LO_SCALE), lb[:],
                    mybir.AluOpType.mult, mybir.AluOpType.add)
                nc.vector.scalar_tensor_tensor(
                    out=res[:], in0=ps_hi[:], scalar=1.0 / W_SCALE,
                    in1=half[:],
                    op0=mybir.AluOpType.mult, op1=mybir.AluOpType.add)
            else:
                nc.vector.tensor_scalar(
                    res[:], ps_hi[:], 1.0 / W_SCALE, lb[:],
                    mybir.AluOpType.mult, mybir.AluOpType.add)
            nc.sync.dma_start(out=out[:, :], in_=res[:])
    nc.finalize()
    return nc


def _shard_inputs(x1, x2, x3, share_feature, A, Ws3, lin_b, idx_h, idx_w,
                  hilo=HILO):
    """Gather crops, pack per-core PE-layout tensors (fp16)."""
    nlo = NCROP if hilo else 0

    xc1 = _crop(x1, idx_h[0], idx_w[0])              # [64, 1024, 49]
    xc2 = _crop(x2, idx_h[1], idx_w[1])
    xcs = _crop(share_feature, idx_h[3], idx_w[3])
    Aq1 = A[:, 0:7, 0:7].reshape(1024, 49)
    Aq2 = A[:, 7:14, 0:7].reshape(1024, 49)
    Aq4 = A[:, 7:14, 7:14].reshape(1024, 49)
    x3f = np.asarray(x3, np.float32).reshape(NB, 1280 * 784)
    w3f = Ws3.reshape(1280 * 784)

    in_maps = []
    for m in range(NCORES):
        cs = slice(m * 128, (m + 1) * 128)
        e0, e1 = m * 160 * 784, (m + 1) * 160 * 784

        # Per-core flat element stream: crops (ch-major) ++ x3 slice.
        cropx = np.concatenate(
            [xc1[:, cs], xc2[:, cs], xcs[:, cs]], axis=2)    # [64, 128, 147]
        xall = np.concatenate(
            [cropx.reshape(NB, 128 * NCROP), x3f[:, e0:e1]], axis=1)
        # chunks: [64, 1127, 128] -> [128, 1127, 64]
        xpe = np.ascontiguousarray(
            xall.reshape(NB, NCHUNK, 128).transpose(2, 1, 0),
            dtype=np.float16)

        cropw = np.concatenate(
            [Aq1[cs], Aq2[cs], Aq4[cs]], axis=1)             # [128, 147]
        wall = np.concatenate(
            [cropw.reshape(128 * NCROP), w3f[e0:e1]]) * W_SCALE
        whi = wall.reshape(NCHUNK, 128).T.astype(np.float16)  # [128, 1127]
        if nlo:
            wlo = ((wall[:128 * NCROP]
                    - whi.T.reshape(NCHUNK * 128)[:128 * NCROP]
                        .astype(np.float64))
                   * LO_SCALE).reshape(nlo, 128).T.astype(np.float16)
            wpe = np.ascontiguousarray(
                np.concatenate([whi, wlo], axis=1), dtype=np.float16)
        else:
            wpe = np.ascontiguousarray(whi, dtype=np.float16)

        linb = np.array([[lin_b[0] if m == 0 else 0.0]], np.float32)
        in_maps.append({'xpe': xpe, 'wpe': wpe, 'linb': linb})
    return in_maps


def _prepare(inputs):
    """Fold weights + shard; returns (nc, in_maps)."""
    A, Ws3 = _build_fold(
        np.asarray(inputs['c_w']), np.asarray(inputs['conv3d_w']),
        np.asarray(inputs['lin_w']), np.asarray(inputs['lin_b']),
        np.asarray(inputs['idx_h']), np.asarray(inputs['idx_w']))
    in_maps = _shard_inputs(
        np.asarray(inputs['x1']), np.asarray(inputs['x2']),
        np.asarray(inputs['x3']), np.asarray(inputs['share_feature']),
        A, Ws3, np.asarray(inputs['lin_b']),
        np.asarray(inputs['idx_h']), np.asarray(inputs['idx_w']))
    nc = _build_bass_raw()
    return nc, in_maps


def _ensure_ntff_hook():
    """Make `trace=True` (e.g. BASS_TRACE=1) work under axon even when the
    image's antenv package lacks axon_hooks: register an equivalent module
    backed by the ctypes NTFF hook from trn_agent_boot."""
    import sys
    import types
    try:
        import antenv.axon_hooks  # noqa: F401
        return
    except Exception:
        pass
    try:
        from trn_agent_boot import trn_boot
        hook = trn_boot._ntff_profile_via_ctypes('/opt/axon/libaxon_pjrt.so')
        mod = types.ModuleType('antenv.axon_hooks')
        mod.get_axon_ntff_profile_hook = lambda: hook
        mod.set_axon_ntff_profile_hook = lambda h: None
        sys.modules['antenv.axon_hooks'] = mod
    except Exception:
        pass


def kernel(x1, x2, x3, share_feature, c_w, conv3d_w, lin_w, lin_b,
           idx_h, idx_w):
    _ensure_ntff_hook()
    nc, in_maps = _prepare({
        'x1': x1, 'x2': x2, 'x3': x3, 'share_feature': share_feature,
        'c_w': c_w, 'conv3d_w': conv3d_w, 'lin_w': lin_w, 'lin_b': lin_b,
        'idx_h': idx_h, 'idx_w': idx_w})
    res = run_bass_kernel_spmd(nc, in_maps, core_ids=list(range(NCORES)))
    parts = np.stack([r['out'][0] for r in res.results])      # [8, 64]
    return parts.sum(axis=0, dtype=np.float64).astype(np.float32).reshape(
        NB, 1)


# revision 10
# speedup vs baseline: 1.1669x; 1.0571x over previous
"""Trainium2 Bass kernel for nn_Net_73710228734901.

The network's post-gather graph (concat -> Conv3d -> spatial mean -> Linear)
is entirely linear in the gathered pixels, and the gathers / avg-pool /
1x1-conv are linear in the inputs.  Since the output is only [B, 1], the
whole model collapses to

    out[b] = lin_b + <W1, x1crop[b]> + <W2, x2crop[b]> + <W4, sharecrop[b]>
                   + <W3, x3[b]>

with fixed per-element weight tensors computed (cheaply, on host) from
c_w / conv3d_w / lin_w / idx_h / idx_w.  Only the 7x7 per-channel crop
windows of x1/x2/share carry nonzero weight, so the host packs just those
49 of 196 positions per channel (pure indexing); x3's weights are dense
(the 1x1 conv mixes all channels), so all of x3 streams.

Device kernel (per core, channel-sharded 8 ways): the whole reduction runs
on the TensorEngine as a chain of [128,1]^T @ [128,64] matvec matmuls
accumulating in PSUM.  Chunk k holds 128 consecutive elements of the
core's (crops ++ x3) stream across partitions, for all 64 batches; lhsT
is the matching fp16 weight column.  Products are exact (fp16 in, fp32
accumulate).  The 147 crop chunks additionally run a second matmul with
the fp16 *residual* weight column into a second PSUM bank, recovering
~fp32 weight precision for the crops at no extra DMA cost.  ~35 ns per
chunk on the PE; DVE/ACT stay idle.

Per-core HBM traffic = 18.5 MB of fp16 activations, the memory roofline
for this problem; the PE chain (~45 us) hides entirely under the DMA
stream (~53 us).
"""

import numpy as np

import concourse.bacc as bacc
import concourse.mybir as mybir
from concourse.bass_utils import run_bass_kernel_spmd
from concourse.tile import TileContext

NCORES = 8
NB = 64            # full batch, all on every core (channel sharding)
NCROP = 147        # 3 * 49 crop elems per (partition, batch)
NCH3 = 980         # x3 elems per partition: 160 ch * 784 pos / 128
NCHUNK = NCROP + NCH3   # 1127 PE chunks of 128 elements
CPT = 70           # PE chunks per DMA tile
PEBUFS = 4         # PE x-tile buffer depth
HILO = True        # double-fp16 weights for the crop chunks
W_SCALE = 1024.0   # weights pre-scaled so fp16 values avoid subnormals
LO_SCALE = 2048.0  # extra scale on the residual (lo) weight columns

_F32 = mybir.dt.float32
_F16 = mybir.dt.float16


def _build_fold(c_w, conv3d_w, lin_w, lin_b, idx_h, idx_w):
    """Collapse conv3d+mean+linear into per-element weights (float64 host).

    Returns A: [1024, 14, 14] quadrant weights (applied to the gathered
    crops directly) and Ws3: [1280, 784] dense weights for raw x3.
    """
    c_w = c_w.astype(np.float64)
    conv3d_w = conv3d_w.astype(np.float64)
    lin_w = lin_w.astype(np.float64)

    # W2[c = i*64+dd, kh, kw] = sum_{o,d,kd: 3d-4+kd=dd} lin_w[o*24+d]
    #                           * conv3d_w[o,i,kd,kh,kw]
    W2 = np.zeros((1024, 3, 3), np.float64)
    o_idx = np.arange(32) * 24
    i_idx = np.arange(16) * 64
    for d in range(24):
        for kd in range(3):
            dd = 3 * d - 4 + kd
            if 0 <= dd < 64:
                W2[i_idx + dd] += np.einsum(
                    'o,oikl->ikl', lin_w[o_idx + d, 0], conv3d_w[:, :, kd])

    # Mean over the 14x14 conv output folds each (kh,kw) tap into a
    # border mask.
    M = np.zeros((3, 3, 14, 14), np.float64)
    rng = {0: (0, 13), 1: (0, 14), 2: (1, 14)}
    for kh in range(3):
        for kw in range(3):
            r0, r1 = rng[kh]
            c0, c1 = rng[kw]
            M[kh, kw, r0:r1, c0:c1] = 1.0
    A = np.einsum('ckl,klrs->crs', W2, M) / 196.0   # [1024, 14, 14]

    # x3 path: scatter quadrant 3's 7x7 weights to the pooled 14x14 grid
    # at the per-channel crop offset, pull back through the 1x1 conv and
    # the transposed avg_pool2d(5, stride 2, pad 2).
    Aq3 = A[:, 0:7, 7:14]
    Ws3c = np.zeros((1024, 14, 14), np.float64)
    ci = np.arange(1024)[:, None, None]
    ri = (idx_h[2][:, None] + np.arange(7))[:, :, None]
    wi = (idx_w[2][:, None] + np.arange(7))[:, None, :]
    Ws3c[ci, ri, wi] = Aq3
    Wpool = np.einsum('oc,ohw->chw', c_w, Ws3c)     # [1280, 14, 14]
    Ws3 = np.zeros((1280, 28, 28), np.float64)
    for dh in range(-2, 3):
        for dw in range(-2, 3):
            hs = [h for h in range(14) if 0 <= 2 * h + dh < 28]
            ws = [w for w in range(14) if 0 <= 2 * w + dw < 28]
            H = [2 * h + dh for h in hs]
            W_ = [2 * w + dw for w in ws]
            Ws3[:, np.ix_(H, W_)[0], np.ix_(H, W_)[1]] += \
                Wpool[:, np.ix_(hs, ws)[0], np.ix_(hs, ws)[1]] / 25.0

    return A, Ws3.reshape(1280, 784)


def _crop(x, ih, iw):
    """Gather per-channel 7x7 crops: [B,1024,14,14] -> [B,1024,49]."""
    c = np.arange(x.shape[1])[None, :, None, None]
    r = (ih[:, None] + np.arange(7))[None, :, :, None]
    w = (iw[:, None] + np.arange(7))[None, :, None, :]
    return x[:, c, r, w].reshape(x.shape[0], x.shape[1], 49)


def _tiles(cpt=CPT):
    """Chunk ranges per DMA transfer; runt tile last to shorten the tail."""
    ts = []
    k = 0
    while k < NCHUNK:
        k1 = min(k + cpt, NCHUNK)
        ts.append((k, k1))
        k = k1
    return ts


def _build_bass_raw(cpt=CPT, hilo=HILO):
    """Raw (non-Tile) build: whole input resident in SBUF, manual sems.

    Avoids the Tile framework's ~8 us prologue (pool/sem setup) and ~10 us
    epilogue (sem recycling ladder + multi-round exit barriers): one
    counting semaphore tracks the in-order data-DMA stream, the PE chain
    waits per tile, and the program ends right after the output DMA
    confirms.
    """
    nlo = NCROP if hilo else 0
    tiles = _tiles(cpt)

    nc = bacc.Bacc("TRN2")
    xpe = nc.dram_tensor("xpe", [128, NCHUNK, NB], _F16, kind="ExternalInput")
    wpe = nc.dram_tensor("wpe", [128, NCHUNK + nlo], _F16,
                         kind="ExternalInput")
    linb = nc.dram_tensor("linb", [1, 1], _F32, kind="ExternalInput")
    out = nc.dram_tensor("out", [1, NB], _F32, kind="ExternalOutput")

    xt = nc.alloc_sbuf_tensor("xt", [128, NCHUNK * NB], _F16)
    wp = nc.alloc_sbuf_tensor("wp", [128, NCHUNK + nlo], _F16)
    lbt = nc.alloc_sbuf_tensor("lbt", [1, 1], _F32)
    half = nc.alloc_sbuf_tensor("half", [1, NB], _F32)
    res = nc.alloc_sbuf_tensor("res", [1, NB], _F32)
    ps_hi = nc.alloc_psum_tensor("ps_hi", [1, NB], _F32)
    ps_lo = nc.alloc_psum_tensor("ps_lo", [1, NB], _F32)

    dsem = nc.alloc_semaphore("dsem")
    wsem = nc.alloc_semaphore("wsem")
    csem = nc.alloc_semaphore("csem")
    osem = nc.alloc_semaphore("osem")

    # Weights + bias from the scalar queue; the data stream owns sync.
    nc.scalar.dma_start(wp[:, :], wpe[:, :]).then_inc(wsem, 16)
    nc.scalar.dma_start(lbt[:, :], linb[:, :]).then_inc(wsem, 16)
    for (k0, k1) in tiles:
        nc.sync.dma_start(
            xt[:, k0 * NB:k1 * NB], xpe[:, k0:k1, :]).then_inc(dsem, 16)

    # PE chain: per tile, wait for its (in-order) completion count.
    nc.tensor.wait_ge(wsem, 32)
    for t, (k0, k1) in enumerate(tiles):
        nc.tensor.wait_ge(dsem, 16 * (t + 1))
        for k in range(k0, k1):
            rhs = xt[:, k * NB:(k + 1) * NB]
            mm = nc.tensor.matmul(
                ps_hi[:, :], lhsT=wp[:, k:k + 1], rhs=rhs,
                start=(k == 0), stop=(k == NCHUNK - 1))
            if k == NCHUNK - 1:
                mm.then_inc(csem, 1)
            if k < nlo:
                mm2 = nc.tensor.matmul(
                    ps_lo[:, :], lhsT=wp[:, NCHUNK + k:NCHUNK + k + 1],
                    rhs=rhs, start=(k == 0), stop=(k == nlo - 1))
                if k == nlo - 1:
                    mm2.then_inc(csem, 1)

    # Combine on DVE, then the output DMA; hold the sync queue until the
    # write is confirmed so execution cannot retire early.
    nc.vector.wait_ge(csem, 2 if nlo else 1)
    if nlo:
        nc.vector.tensor_scalar(
            half[:, :], ps_lo[:, :], 1.0 / (W_SCALE * LO_SCALE), lbt[:, :],
            mybir.AluOpType.mult, mybir.AluOpType.add)
        nc.vector.scalar_tensor_tensor(
            out=res[:, :], in0=ps_hi[:, :], scalar=1.0 / W_SCALE,
            in1=half[:, :], op0=mybir.AluOpType.mult,
            op1=mybir.AluOpType.add).then_inc(csem, 1)
    else:
        nc.vector.tensor_scalar(
            res[:, :], ps_hi[:, :], 1.0 / W_SCALE, lbt[:, :],
            mybir.AluOpType.mult, mybir.AluOpType.add).then_inc(csem, 1)
    nc.sync.wait_ge(csem, 3 if nlo else 2)
    nc.sync.dma_start(out[:, :], res[:, :]).then_inc(osem, 16)
    nc.sync.wait_ge(osem, 16)
    # Self-cleaning epilogue: zero our sems once everything is quiescent so
    # the NEXT execution of this program starts clean with no start-side
    # barrier (sem state persists across executions of a loaded NEFF).
    nc.gpsimd.wait_ge(osem, 16)
    for s in (dsem, wsem, csem, osem):
        nc.gpsimd.sem_clear(s)
    nc.finalize()
    return nc


def _build_bass(cpt=CPT, pebufs=PEBUFS, hilo=HILO):
    nlo = NCROP if hilo else 0
    ntiles = (NCHUNK + cpt - 1) // cpt

    nc = bacc.Bacc("TRN2")
    xpe = nc.dram_tensor("xpe", [128, NCHUNK, NB], _F16, kind="ExternalInput")
    wpe = nc.dram_tensor("wpe", [128, NCHUNK + nlo], _F16,
                         kind="ExternalInput")
    linb = nc.dram_tensor("linb", [1, 1], _F32, kind="ExternalInput")
    out = nc.dram_tensor("out", [1, NB], _F32, kind="ExternalOutput")

    with TileContext(nc) as tc:
        with (
            tc.tile_pool(name="cpool", bufs=1) as cpool,
            tc.tile_pool(name="pepool", bufs=pebufs) as pepool,
            tc.tile_pool(name="apool", bufs=1) as apool,
            tc.tile_pool(name="ppool", bufs=2, space="PSUM") as ppool,
        ):
            # Weights + bias issue from the scalar queue so the data
            # stream owns the sync queue from t=0.
            wp = cpool.tile([128, NCHUNK + nlo], _F16)
            nc.scalar.dma_start(out=wp[:], in_=wpe[:, :])
            lb = cpool.tile([1, 1], _F32)
            nc.scalar.dma_start(out=lb[:], in_=linb[:, :])

            ps_hi = ppool.tile([1, NB], _F32)
            ps_lo = ppool.tile([1, NB], _F32)
            for t in range(ntiles):
                k0 = t * cpt
                k1 = min(k0 + cpt, NCHUNK)
                pt = pepool.tile([128, (k1 - k0) * NB], _F16, tag="pt")
                nc.sync.dma_start(out=pt[:], in_=xpe[:, k0:k1, :])
                for k in range(k0, k1):
                    rhs = pt[:, (k - k0) * NB:(k - k0 + 1) * NB]
                    nc.tensor.matmul(
                        ps_hi[:], lhsT=wp[:, k:k + 1], rhs=rhs,
                        start=(k == 0), stop=(k == NCHUNK - 1))
                    if k < nlo:
                        nc.tensor.matmul(
                            ps_lo[:], lhsT=wp[:, NCHUNK + k:NCHUNK + k + 1],
                            rhs=rhs, start=(k == 0), stop=(k == nlo - 1))

            # Combine the two PSUM registers, undo the weight pre-scales,
            # add lin_b.
            res = apool.tile([1, NB], _F32)
            if nlo:
                half = apool.tile([1, NB], _F32)
                nc.vector.tensor_scalar(
                    half[:], ps_lo[:], 1.0 / (W_SCALE * LO_SCALE), lb[:],
                    mybir.AluOpType.mult, mybir.AluOpType.add)
                nc.vector.scalar_tensor_tensor(
                    out=res[:], in0=ps_hi[:], scalar=1.0 / W_SCALE,
                    in1=half[:],
                    op0=mybir.AluOpType.mult, op1=mybir.AluOpType.add)
            else:
                nc.vector.tensor_scalar(
                    res[:], ps_hi[:], 1.0 / W_SCALE, lb[:],
                    mybir.AluOpType.mult, mybir.AluOpType.add)
            nc.sync.dma_start(out=out[:, :], in_=res[:])
    nc.finalize()
    return nc


def _shard_inputs(x1, x2, x3, share_feature, A, Ws3, lin_b, idx_h, idx_w,
                  hilo=HILO):
    """Gather crops, pack per-core PE-layout tensors (fp16)."""
    nlo = NCROP if hilo else 0

    xc1 = _crop(x1, idx_h[0], idx_w[0])              # [64, 1024, 49]
    xc2 = _crop(x2, idx_h[1], idx_w[1])
    xcs = _crop(share_feature, idx_h[3], idx_w[3])
    Aq1 = A[:, 0:7, 0:7].reshape(1024, 49)
    Aq2 = A[:, 7:14, 0:7].reshape(1024, 49)
    Aq4 = A[:, 7:14, 7:14].reshape(1024, 49)
    x3f = np.asarray(x3, np.float32).reshape(NB, 1280 * 784)
    w3f = Ws3.reshape(1280 * 784)

    in_maps = []
    for m in range(NCORES):
        cs = slice(m * 128, (m + 1) * 128)
        e0, e1 = m * 160 * 784, (m + 1) * 160 * 784

        # Per-core flat element stream: crops (ch-major) ++ x3 slice.
        cropx = np.concatenate(
            [xc1[:, cs], xc2[:, cs], xcs[:, cs]], axis=2)    # [64, 128, 147]
        xall = np.concatenate(
            [cropx.reshape(NB, 128 * NCROP), x3f[:, e0:e1]], axis=1)
        # chunks: [64, 1127, 128] -> [128, 1127, 64]
        xpe = np.ascontiguousarray(
            xall.reshape(NB, NCHUNK, 128).transpose(2, 1, 0),
            dtype=np.float16)

        cropw = np.concatenate(
            [Aq1[cs], Aq2[cs], Aq4[cs]], axis=1)             # [128, 147]
        wall = np.concatenate(
            [cropw.reshape(128 * NCROP), w3f[e0:e1]]) * W_SCALE
        whi = wall.reshape(NCHUNK, 128).T.astype(np.float16)  # [128, 1127]
        if nlo:
            wlo = ((wall[:128 * NCROP]
                    - whi.T.reshape(NCHUNK * 128)[:128 * NCROP]
                        .astype(np.float64))
                   * LO_SCALE).reshape(nlo, 128).T.astype(np.float16)
            wpe = np.ascontiguousarray(
                np.concatenate([whi, wlo], axis=1), dtype=np.float16)
        else:
            wpe = np.ascontiguousarray(whi, dtype=np.float16)

        linb = np.array([[lin_b[0] if m == 0 else 0.0]], np.float32)
        in_maps.append({'xpe': xpe, 'wpe': wpe, 'linb': linb})
    return in_maps


def _prepare(inputs):
    """Fold weights + shard; returns (nc, in_maps)."""
    A, Ws3 = _build_fold(
        np.asarray(inputs['c_w']), np.asarray(inputs['conv3d_w']),
        np.asarray(inputs['lin_w']), np.asarray(inputs['lin_b']),
        np.asarray(inputs['idx_h']), np.asarray(inputs['idx_w']))
    in_maps = _shard_inputs(
        np.asarray(inputs['x1']), np.asarray(inputs['x2']),
        np.asarray(inputs['x3']), np.asarray(inputs['share_feature']),
        A, Ws3, np.asarray(inputs['lin_b']),
        np.asarray(inputs['idx_h']), np.asarray(inputs['idx_w']))
    nc = _build_bass_raw()
    return nc, in_maps


def _ensure_ntff_hook():
    """Make `trace=True` (e.g. BASS_TRACE=1) work under axon even when the
    image's antenv package lacks axon_hooks: register an equivalent module
    backed by the ctypes NTFF hook from trn_agent_boot."""
    import sys
    import types
    try:
        import antenv.axon_hooks  # noqa: F401
        return
    except Exception:
        pass
    try:
        from trn_agent_boot import trn_boot
        hook = trn_boot._ntff_profile_via_ctypes('/opt/axon/libaxon_pjrt.so')
        mod = types.ModuleType('antenv.axon_hooks')
        mod.get_axon_ntff_profile_hook = lambda: hook
        mod.set_axon_ntff_profile_hook = lambda h: None
        sys.modules['antenv.axon_hooks'] = mod
    except Exception:
        pass


def kernel(x1, x2, x3, share_feature, c_w, conv3d_w, lin_w, lin_b,
           idx_h, idx_w):
    _ensure_ntff_hook()
    nc, in_maps = _prepare({
        'x1': x1, 'x2': x2, 'x3': x3, 'share_feature': share_feature,
        'c_w': c_w, 'conv3d_w': conv3d_w, 'lin_w': lin_w, 'lin_b': lin_b,
        'idx_h': idx_h, 'idx_w': idx_w})
    res = run_bass_kernel_spmd(nc, in_maps, core_ids=list(range(NCORES)))
    parts = np.stack([r['out'][0] for r in res.results])      # [8, 64]
    return parts.sum(axis=0, dtype=np.float64).astype(np.float32).reshape(
        NB, 1)


# revision 15
# speedup vs baseline: 1.1976x; 1.0263x over previous
"""Trainium2 Bass kernel for nn_Net_73710228734901.

The network's post-gather graph (concat -> Conv3d -> spatial mean -> Linear)
is entirely linear in the gathered pixels, and the gathers / avg-pool /
1x1-conv are linear in the inputs.  Since the output is only [B, 1], the
whole model collapses to

    out[b] = lin_b + <W1, x1crop[b]> + <W2, x2crop[b]> + <W4, sharecrop[b]>
                   + <W3, x3[b]>

with fixed per-element weight tensors computed (cheaply, on host) from
c_w / conv3d_w / lin_w / idx_h / idx_w.  Only the 7x7 per-channel crop
windows of x1/x2/share carry nonzero weight, so the host packs just those
49 of 196 positions per channel (pure indexing); x3's weights are dense
(the 1x1 conv mixes all channels), so all of x3 streams.

Device kernel (per core, channel-sharded 8 ways): the whole reduction runs
on the TensorEngine as a chain of [128,1]^T @ [128,64] matvec matmuls
accumulating in PSUM.  Chunk k holds 128 consecutive elements of the
core's (crops ++ x3) stream across partitions, for all 64 batches; lhsT
is the matching fp16 weight column.  Products are exact (fp16 in, fp32
accumulate).  The 147 crop chunks additionally run a second matmul with
the fp16 *residual* weight column into a second PSUM bank, recovering
~fp32 weight precision for the crops at no extra DMA cost.  ~35 ns per
chunk on the PE; DVE/ACT stay idle.

Per-core HBM traffic = 18.5 MB of fp16 activations, the memory roofline
for this problem; the PE chain (~45 us) hides entirely under the DMA
stream (~53 us).
"""

import numpy as np

import concourse.bacc as bacc
import concourse.mybir as mybir
from concourse.bass_utils import run_bass_kernel_spmd
from concourse.tile import TileContext

NCORES = 8
NB = 64            # full batch, all on every core (channel sharding)
NCROP = 147        # 3 * 49 crop elems per (partition, batch)
NCH3 = 980         # x3 elems per partition: 160 ch * 784 pos / 128
NCHUNK = NCROP + NCH3   # 1127 PE chunks of 128 elements
CPT = 70           # PE chunks per DMA tile
PEBUFS = 4         # PE x-tile buffer depth
HILO = True        # double-fp16 weights for the crop chunks
W_SCALE = 1024.0   # weights pre-scaled so fp16 values avoid subnormals
LO_SCALE = 2048.0  # extra scale on the residual (lo) weight columns

_F32 = mybir.dt.float32
_F16 = mybir.dt.float16


def _build_fold(c_w, conv3d_w, lin_w, lin_b, idx_h, idx_w):
    """Collapse conv3d+mean+linear into per-element weights (float64 host).

    Returns A: [1024, 14, 14] quadrant weights (applied to the gathered
    crops directly) and Ws3: [1280, 784] dense weights for raw x3.
    """
    c_w = c_w.astype(np.float64)
    conv3d_w = conv3d_w.astype(np.float64)
    lin_w = lin_w.astype(np.float64)

    # W2[c = i*64+dd, kh, kw] = sum_{o,d,kd: 3d-4+kd=dd} lin_w[o*24+d]
    #                           * conv3d_w[o,i,kd,kh,kw]
    W2 = np.zeros((1024, 3, 3), np.float64)
    o_idx = np.arange(32) * 24
    i_idx = np.arange(16) * 64
    for d in range(24):
        for kd in range(3):
            dd = 3 * d - 4 + kd
            if 0 <= dd < 64:
                W2[i_idx + dd] += np.einsum(
                    'o,oikl->ikl', lin_w[o_idx + d, 0], conv3d_w[:, :, kd])

    # Mean over the 14x14 conv output folds each (kh,kw) tap into a
    # border mask.
    M = np.zeros((3, 3, 14, 14), np.float64)
    rng = {0: (0, 13), 1: (0, 14), 2: (1, 14)}
    for kh in range(3):
        for kw in range(3):
            r0, r1 = rng[kh]
            c0, c1 = rng[kw]
            M[kh, kw, r0:r1, c0:c1] = 1.0
    A = np.einsum('ckl,klrs->crs', W2, M) / 196.0   # [1024, 14, 14]

    # x3 path: scatter quadrant 3's 7x7 weights to the pooled 14x14 grid
    # at the per-channel crop offset, pull back through the 1x1 conv and
    # the transposed avg_pool2d(5, stride 2, pad 2).
    Aq3 = A[:, 0:7, 7:14]
    Ws3c = np.zeros((1024, 14, 14), np.float64)
    ci = np.arange(1024)[:, None, None]
    ri = (idx_h[2][:, None] + np.arange(7))[:, :, None]
    wi = (idx_w[2][:, None] + np.arange(7))[:, None, :]
    Ws3c[ci, ri, wi] = Aq3
    Wpool = np.einsum('oc,ohw->chw', c_w, Ws3c)     # [1280, 14, 14]
    Ws3 = np.zeros((1280, 28, 28), np.float64)
    for dh in range(-2, 3):
        for dw in range(-2, 3):
            hs = [h for h in range(14) if 0 <= 2 * h + dh < 28]
            ws = [w for w in range(14) if 0 <= 2 * w + dw < 28]
            H = [2 * h + dh for h in hs]
            W_ = [2 * w + dw for w in ws]
            Ws3[:, np.ix_(H, W_)[0], np.ix_(H, W_)[1]] += \
                Wpool[:, np.ix_(hs, ws)[0], np.ix_(hs, ws)[1]] / 25.0

    return A, Ws3.reshape(1280, 784)


def _crop(x, ih, iw):
    """Gather per-channel 7x7 crops: [B,1024,14,14] -> [B,1024,49]."""
    c = np.arange(x.shape[1])[None, :, None, None]
    r = (ih[:, None] + np.arange(7))[None, :, :, None]
    w = (iw[:, None] + np.arange(7))[None, :, None, :]
    return x[:, c, r, w].reshape(x.shape[0], x.shape[1], 49)


def _tiles(cpt=CPT):
    """Chunk ranges per DMA transfer.

    Big tiles for the bulk of the stream, then a tapered tail: the PE runs
    ~2 us behind the stream (DMA completion-receipt latency), so shrinking
    the final transfers minimizes the serial compute left after the last
    semaphore fires.
    """
    sizes = [140] * 7 + [70, 35, 21, 14, 4, 2, 1]
    assert sum(sizes) == NCHUNK
    ts = []
    k = 0
    for s in sizes:
        ts.append((k, k + s))
        k += s
    return ts


def _build_bass_raw(cpt=CPT, hilo=HILO):
    """Raw (non-Tile) build: whole input resident in SBUF, manual sems.

    Avoids the Tile framework's ~8 us prologue (pool/sem setup) and ~10 us
    epilogue (sem recycling ladder + multi-round exit barriers): one
    counting semaphore tracks the in-order data-DMA stream, the PE chain
    waits per tile, and the program ends right after the output DMA
    confirms.
    """
    nlo = NCROP if hilo else 0
    tiles = _tiles(cpt)

    nc = bacc.Bacc("TRN2")
    xpe = nc.dram_tensor("xpe", [128, NCHUNK, NB], _F16, kind="ExternalInput")
    wpe = nc.dram_tensor("wpe", [128, NCHUNK + nlo], _F16,
                         kind="ExternalInput")
    linb = nc.dram_tensor("linb", [1, 1], _F32, kind="ExternalInput")
    out = nc.dram_tensor("out", [1, NB], _F32, kind="ExternalOutput")

    xt = nc.alloc_sbuf_tensor("xt", [128, NCHUNK * NB], _F16)
    wp = nc.alloc_sbuf_tensor("wp", [128, NCHUNK + nlo], _F16)
    lbt = nc.alloc_sbuf_tensor("lbt", [1, 1], _F32)
    half = nc.alloc_sbuf_tensor("half", [1, NB], _F32)
    res = nc.alloc_sbuf_tensor("res", [1, NB], _F32)
    ps_hi = nc.alloc_psum_tensor("ps_hi", [1, NB], _F32)
    ps_lo = nc.alloc_psum_tensor("ps_lo", [1, NB], _F32)

    dsem = nc.alloc_semaphore("dsem")
    wsem = nc.alloc_semaphore("wsem")
    csem = nc.alloc_semaphore("csem")
    osem = nc.alloc_semaphore("osem")

    # Weights + bias from the scalar queue; the data stream owns sync.
    nc.scalar.dma_start(wp[:, :], wpe[:, :]).then_inc(wsem, 16)
    nc.scalar.dma_start(lbt[:, :], linb[:, :]).then_inc(wsem, 16)
    for (k0, k1) in tiles:
        nc.sync.dma_start(
            xt[:, k0 * NB:k1 * NB], xpe[:, k0:k1, :]).then_inc(dsem, 16)

    # PE chain: per tile, wait for its (in-order) completion count.
    nc.tensor.wait_ge(wsem, 32)
    for t, (k0, k1) in enumerate(tiles):
        nc.tensor.wait_ge(dsem, 16 * (t + 1))
        for k in range(k0, k1):
            rhs = xt[:, k * NB:(k + 1) * NB]
            mm = nc.tensor.matmul(
                ps_hi[:, :], lhsT=wp[:, k:k + 1], rhs=rhs,
                start=(k == 0), stop=(k == NCHUNK - 1))
            if k == NCHUNK - 1:
                mm.then_inc(csem, 1)
            if k < nlo:
                mm2 = nc.tensor.matmul(
                    ps_lo[:, :], lhsT=wp[:, NCHUNK + k:NCHUNK + k + 1],
                    rhs=rhs, start=(k == 0), stop=(k == nlo - 1))
                if k == nlo - 1:
                    mm2.then_inc(csem, 1)

    # Combine (ACT folds the lo register + bias, DVE adds the hi register),
    # then the output DMA.  No trailing waits: the NRT postamble fences DMA
    # completion and zeroes the whole semaphore file, so the next execution
    # starts clean.
    if nlo:
        nc.vector.wait_ge(csem, 2)
        nc.vector.tensor_scalar(
            half[:, :], ps_lo[:, :], 1.0 / (W_SCALE * LO_SCALE), lbt[:, :],
            mybir.AluOpType.mult, mybir.AluOpType.add)
        nc.vector.scalar_tensor_tensor(
            out=res[:, :], in0=ps_hi[:, :], scalar=1.0 / W_SCALE,
            in1=half[:, :], op0=mybir.AluOpType.mult,
            op1=mybir.AluOpType.add).then_inc(csem, 1)
        nc.sync.wait_ge(csem, 3)
    else:
        nc.vector.wait_ge(csem, 1)
        nc.vector.tensor_scalar(
            res[:, :], ps_hi[:, :], 1.0 / W_SCALE, lbt[:, :],
            mybir.AluOpType.mult, mybir.AluOpType.add).then_inc(csem, 1)
        nc.sync.wait_ge(csem, 2)
    nc.sync.dma_start(out[:, :], res[:, :]).then_inc(osem, 16)
    nc.finalize()
    return nc


def _build_bass(cpt=CPT, pebufs=PEBUFS, hilo=HILO):
    nlo = NCROP if hilo else 0
    ntiles = (NCHUNK + cpt - 1) // cpt

    nc = bacc.Bacc("TRN2")
    xpe = nc.dram_tensor("xpe", [128, NCHUNK, NB], _F16, kind="ExternalInput")
    wpe = nc.dram_tensor("wpe", [128, NCHUNK + nlo], _F16,
                         kind="ExternalInput")
    linb = nc.dram_tensor("linb", [1, 1], _F32, kind="ExternalInput")
    out = nc.dram_tensor("out", [1, NB], _F32, kind="ExternalOutput")

    with TileContext(nc) as tc:
        with (
            tc.tile_pool(name="cpool", bufs=1) as cpool,
            tc.tile_pool(name="pepool", bufs=pebufs) as pepool,
            tc.tile_pool(name="apool", bufs=1) as apool,
            tc.tile_pool(name="ppool", bufs=2, space="PSUM") as ppool,
        ):
            # Weights + bias issue from the scalar queue so the data
            # stream owns the sync queue from t=0.
            wp = cpool.tile([128, NCHUNK + nlo], _F16)
            nc.scalar.dma_start(out=wp[:], in_=wpe[:, :])
            lb = cpool.tile([1, 1], _F32)
            nc.scalar.dma_start(out=lb[:], in_=linb[:, :])

            ps_hi = ppool.tile([1, NB], _F32)
            ps_lo = ppool.tile([1, NB], _F32)
            for t in range(ntiles):
                k0 = t * cpt
                k1 = min(k0 + cpt, NCHUNK)
                pt = pepool.tile([128, (k1 - k0) * NB], _F16, tag="pt")
                nc.sync.dma_start(out=pt[:], in_=xpe[:, k0:k1, :])
                for k in range(k0, k1):
                    rhs = pt[:, (k - k0) * NB:(k - k0 + 1) * NB]
                    nc.tensor.matmul(
                        ps_hi[:], lhsT=wp[:, k:k + 1], rhs=rhs,
                        start=(k == 0), stop=(k == NCHUNK - 1))
                    if k < nlo:
                        nc.tensor.matmul(
                            ps_lo[:], lhsT=wp[:, NCHUNK + k:NCHUNK + k + 1],
                            rhs=rhs, start=(k == 0), stop=(k == nlo - 1))

            # Combine the two PSUM registers, undo the weight pre-scales,
            # add lin_b.
            res = apool.tile([1, NB], _F32)
            if nlo:
                half = apool.tile([1, NB], _F32)
                nc.vector.tensor_scalar(
                    half[:], ps_lo[:], 1.0 / (W_SCALE * LO_SCALE), lb[:],
                    mybir.AluOpType.mult, mybir.AluOpType.add)
                nc.vector.scalar_tensor_tensor(
                    out=res[:], in0=ps_hi[:], scalar=1.0 / W_SCALE,
                    in1=half[:],
                    op0=mybir.AluOpType.mult, op1=mybir.AluOpType.add)
            else:
                nc.vector.tensor_scalar(
                    res[:], ps_hi[:], 1.0 / W_SCALE, lb[:],
                    mybir.AluOpType.mult, mybir.AluOpType.add)
            nc.sync.dma_start(out=out[:, :], in_=res[:])
    nc.finalize()
    return nc


def _shard_inputs(x1, x2, x3, share_feature, A, Ws3, lin_b, idx_h, idx_w,
                  hilo=HILO):
    """Gather crops, pack per-core PE-layout tensors (fp16)."""
    nlo = NCROP if hilo else 0

    xc1 = _crop(x1, idx_h[0], idx_w[0])              # [64, 1024, 49]
    xc2 = _crop(x2, idx_h[1], idx_w[1])
    xcs = _crop(share_feature, idx_h[3], idx_w[3])
    Aq1 = A[:, 0:7, 0:7].reshape(1024, 49)
    Aq2 = A[:, 7:14, 0:7].reshape(1024, 49)
    Aq4 = A[:, 7:14, 7:14].reshape(1024, 49)
    x3f = np.asarray(x3, np.float32).reshape(NB, 1280 * 784)
    w3f = Ws3.reshape(1280 * 784)

    in_maps = []
    for m in range(NCORES):
        cs = slice(m * 128, (m + 1) * 128)
        e0, e1 = m * 160 * 784, (m + 1) * 160 * 784

        # Per-core flat element stream: crops (ch-major) ++ x3 slice.
        cropx = np.concatenate(
            [xc1[:, cs], xc2[:, cs], xcs[:, cs]], axis=2)    # [64, 128, 147]
        xall = np.concatenate(
            [cropx.reshape(NB, 128 * NCROP), x3f[:, e0:e1]], axis=1)
        # chunks: [64, 1127, 128] -> [128, 1127, 64]
        xpe = np.ascontiguousarray(
            xall.reshape(NB, NCHUNK, 128).transpose(2, 1, 0),
            dtype=np.float16)

        cropw = np.concatenate(
            [Aq1[cs], Aq2[cs], Aq4[cs]], axis=1)             # [128, 147]
        wall = np.concatenate(
            [cropw.reshape(128 * NCROP), w3f[e0:e1]]) * W_SCALE
        whi = wall.reshape(NCHUNK, 128).T.astype(np.float16)  # [128, 1127]
        if nlo:
            wlo = ((wall[:128 * NCROP]
                    - whi.T.reshape(NCHUNK * 128)[:128 * NCROP]
                        .astype(np.float64))
                   * LO_SCALE).reshape(nlo, 128).T.astype(np.float16)
            wpe = np.ascontiguousarray(
                np.concatenate([whi, wlo], axis=1), dtype=np.float16)
        else:
            wpe = np.ascontiguousarray(whi, dtype=np.float16)

        linb = np.array([[lin_b[0] if m == 0 else 0.0]], np.float32)
        in_maps.append({'xpe': xpe, 'wpe': wpe, 'linb': linb})
    return in_maps


def _prepare(inputs):
    """Fold weights + shard; returns (nc, in_maps)."""
    A, Ws3 = _build_fold(
        np.asarray(inputs['c_w']), np.asarray(inputs['conv3d_w']),
        np.asarray(inputs['lin_w']), np.asarray(inputs['lin_b']),
        np.asarray(inputs['idx_h']), np.asarray(inputs['idx_w']))
    in_maps = _shard_inputs(
        np.asarray(inputs['x1']), np.asarray(inputs['x2']),
        np.asarray(inputs['x3']), np.asarray(inputs['share_feature']),
        A, Ws3, np.asarray(inputs['lin_b']),
        np.asarray(inputs['idx_h']), np.asarray(inputs['idx_w']))
    nc = _build_bass_raw()
    return nc, in_maps


def _ensure_ntff_hook():
    """Make `trace=True` (e.g. BASS_TRACE=1) work under axon even when the
    image's antenv package lacks axon_hooks: register an equivalent module
    backed by the ctypes NTFF hook from trn_agent_boot."""
    import sys
    import types
    try:
        import antenv.axon_hooks  # noqa: F401
        return
    except Exception:
        pass
    try:
        from trn_agent_boot import trn_boot
        hook = trn_boot._ntff_profile_via_ctypes('/opt/axon/libaxon_pjrt.so')
        mod = types.ModuleType('antenv.axon_hooks')
        mod.get_axon_ntff_profile_hook = lambda: hook
        mod.set_axon_ntff_profile_hook = lambda h: None
        sys.modules['antenv.axon_hooks'] = mod
    except Exception:
        pass


def kernel(x1, x2, x3, share_feature, c_w, conv3d_w, lin_w, lin_b,
           idx_h, idx_w):
    _ensure_ntff_hook()
    nc, in_maps = _prepare({
        'x1': x1, 'x2': x2, 'x3': x3, 'share_feature': share_feature,
        'c_w': c_w, 'conv3d_w': conv3d_w, 'lin_w': lin_w, 'lin_b': lin_b,
        'idx_h': idx_h, 'idx_w': idx_w})
    res = run_bass_kernel_spmd(nc, in_maps, core_ids=list(range(NCORES)))
    parts = np.stack([r['out'][0] for r in res.results])      # [8, 64]
    return parts.sum(axis=0, dtype=np.float64).astype(np.float32).reshape(
        NB, 1)


# revision 23
# speedup vs baseline: 1.2592x; 1.0514x over previous
"""Trainium2 Bass kernel for nn_Net_73710228734901.

The network's post-gather graph (concat -> Conv3d -> spatial mean -> Linear)
is entirely linear in the gathered pixels, and the gathers / avg-pool /
1x1-conv are linear in the inputs.  Since the output is only [B, 1], the
whole model collapses to

    out[b] = lin_b + <W1, x1crop[b]> + <W2, x2crop[b]> + <W4, sharecrop[b]>
                   + <W3, x3[b]>

with fixed per-element weight tensors computed (cheaply, on host) from
c_w / conv3d_w / lin_w / idx_h / idx_w.  Only the 7x7 per-channel crop
windows of x1/x2/share carry nonzero weight, so the host packs just those
49 of 196 positions per channel (pure indexing); x3's weights are dense
(the 1x1 conv mixes all channels), so all of x3 streams.

Device kernel (per core, channel-sharded 8 ways): the whole reduction runs
on the TensorEngine as a chain of [128,1]^T @ [128,64] matvec matmuls
accumulating in PSUM.  Chunk k holds 128 consecutive elements of the
core's (crops ++ x3) stream across partitions, for all 64 batches; lhsT
is the matching fp16 weight column.  Products are exact (fp16 in, fp32
accumulate).  The 147 crop chunks additionally run a second matmul with
the fp16 *residual* weight column into a second PSUM bank, recovering
~fp32 weight precision for the crops at no extra DMA cost.  ~35 ns per
chunk on the PE; DVE/ACT stay idle.

Per-core HBM traffic = 18.5 MB of fp16 activations, the memory roofline
for this problem; the PE chain (~45 us) hides entirely under the DMA
stream (~53 us).
"""

import numpy as np

import concourse.bacc as bacc
import concourse.mybir as mybir
from concourse.bass_utils import run_bass_kernel_spmd
from concourse.tile import TileContext

NCORES = 8
NB = 64            # full batch, all on every core (channel sharding)
NCROP = 147        # 3 * 49 crop elems per (partition, batch)
NCH3 = 980         # x3 elems per partition: 160 ch * 784 pos / 128
NCHUNK = NCROP + NCH3   # 1127 PE chunks of 128 elements
CPT = 70           # PE chunks per DMA tile
PEBUFS = 4         # PE x-tile buffer depth
HILO = True        # double-fp16 weights for the crop chunks
W_SCALE = 1024.0   # weights pre-scaled so fp16 values avoid subnormals
LO_SCALE = 2048.0  # extra scale on the residual (lo) weight columns

_F32 = mybir.dt.float32
_F16 = mybir.dt.float16


def _build_fold(c_w, conv3d_w, lin_w, lin_b, idx_h, idx_w):
    """Collapse conv3d+mean+linear into per-element weights (float64 host).

    Returns A: [1024, 14, 14] quadrant weights (applied to the gathered
    crops directly) and Ws3: [1280, 784] dense weights for raw x3.
    """
    c_w = c_w.astype(np.float64)
    conv3d_w = conv3d_w.astype(np.float64)
    lin_w = lin_w.astype(np.float64)

    # W2[c = i*64+dd, kh, kw] = sum_{o,d,kd: 3d-4+kd=dd} lin_w[o*24+d]
    #                           * conv3d_w[o,i,kd,kh,kw]
    W2 = np.zeros((1024, 3, 3), np.float64)
    o_idx = np.arange(32) * 24
    i_idx = np.arange(16) * 64
    for d in range(24):
        for kd in range(3):
            dd = 3 * d - 4 + kd
            if 0 <= dd < 64:
                W2[i_idx + dd] += np.einsum(
                    'o,oikl->ikl', lin_w[o_idx + d, 0], conv3d_w[:, :, kd])

    # Mean over the 14x14 conv output folds each (kh,kw) tap into a
    # border mask.
    M = np.zeros((3, 3, 14, 14), np.float64)
    rng = {0: (0, 13), 1: (0, 14), 2: (1, 14)}
    for kh in range(3):
        for kw in range(3):
            r0, r1 = rng[kh]
            c0, c1 = rng[kw]
            M[kh, kw, r0:r1, c0:c1] = 1.0
    A = np.einsum('ckl,klrs->crs', W2, M) / 196.0   # [1024, 14, 14]

    # x3 path: scatter quadrant 3's 7x7 weights to the pooled 14x14 grid
    # at the per-channel crop offset, pull back through the 1x1 conv and
    # the transposed avg_pool2d(5, stride 2, pad 2).
    Aq3 = A[:, 0:7, 7:14]
    Ws3c = np.zeros((1024, 14, 14), np.float64)
    ci = np.arange(1024)[:, None, None]
    ri = (idx_h[2][:, None] + np.arange(7))[:, :, None]
    wi = (idx_w[2][:, None] + np.arange(7))[:, None, :]
    Ws3c[ci, ri, wi] = Aq3
    Wpool = np.einsum('oc,ohw->chw', c_w, Ws3c)     # [1280, 14, 14]
    Ws3 = np.zeros((1280, 28, 28), np.float64)
    for dh in range(-2, 3):
        for dw in range(-2, 3):
            hs = [h for h in range(14) if 0 <= 2 * h + dh < 28]
            ws = [w for w in range(14) if 0 <= 2 * w + dw < 28]
            H = [2 * h + dh for h in hs]
            W_ = [2 * w + dw for w in ws]
            Ws3[:, np.ix_(H, W_)[0], np.ix_(H, W_)[1]] += \
                Wpool[:, np.ix_(hs, ws)[0], np.ix_(hs, ws)[1]] / 25.0

    return A, Ws3.reshape(1280, 784)


def _crop(x, ih, iw):
    """Gather per-channel 7x7 crops: [B,1024,14,14] -> [B,1024,49]."""
    c = np.arange(x.shape[1])[None, :, None, None]
    r = (ih[:, None] + np.arange(7))[None, :, :, None]
    w = (iw[:, None] + np.arange(7))[None, :, None, :]
    return x[:, c, r, w].reshape(x.shape[0], x.shape[1], 49)


def _tiles(cpt=CPT):
    """Chunk ranges per DMA transfer.

    Big tiles for the bulk of the stream, then a tapered tail: the PE runs
    ~2 us behind the stream (DMA completion-receipt latency), so shrinking
    the final transfers minimizes the serial compute left after the last
    semaphore fires.
    """
    sizes = ([12, 16, 32, 64] + [140] * 6 + [70, 42, 28, 14, 6, 2, 1])
    assert sum(sizes) == NCHUNK
    ts = []
    k = 0
    for s in sizes:
        ts.append((k, k + s))
        k += s
    return ts


def _build_bass_raw(cpt=CPT, hilo=HILO):
    """Raw (non-Tile) build: whole input resident in SBUF, manual sems.

    Avoids the Tile framework's ~8 us prologue (pool/sem setup) and ~10 us
    epilogue (sem recycling ladder + multi-round exit barriers): one
    counting semaphore tracks the in-order data-DMA stream, the PE chain
    waits per tile, and the program ends right after the output DMA
    confirms.
    """
    nlo = NCROP if hilo else 0
    tiles = _tiles(cpt)

    nc = bacc.Bacc("TRN2")
    xpe = nc.dram_tensor("xpe", [128, NCHUNK, NB], _F16, kind="ExternalInput")
    wpe = nc.dram_tensor("wpe", [128, NCHUNK + nlo], _F16,
                         kind="ExternalInput")
    linb = nc.dram_tensor("linb", [1, 1], _F32, kind="ExternalInput")
    if nlo:
        scl = nc.dram_tensor("scl", [2, 1], _F32, kind="ExternalInput")
    out = nc.dram_tensor("out", [1, NB], _F32, kind="ExternalOutput")

    xt = nc.alloc_sbuf_tensor("xt", [128, NCHUNK * NB], _F16)
    wp = nc.alloc_sbuf_tensor("wp", [128, NCHUNK + nlo], _F16)
    lbt = nc.alloc_sbuf_tensor("lbt", [1, 1], _F32)
    res = nc.alloc_sbuf_tensor("res", [1, NB], _F32)
    # Row 0 accumulates the hi products over all chunks; row 1 the lo
    # (residual) products over the first nlo chunks.
    ps = nc.alloc_psum_tensor("ps", [2, NB], _F32)
    if nlo:
        sclt = nc.alloc_sbuf_tensor("sclt", [2, 1], _F32)
        sb2 = nc.alloc_sbuf_tensor("sb2", [2, NB], _F32)
        ps2 = nc.alloc_psum_tensor("ps2", [1, NB], _F32)

    # One semaphore per data transfer: a single counting sem is racy with
    # unequal transfer sizes (the 16 SDMA engines complete their slices
    # independently, so later small transfers can over-count before an
    # earlier slice lands).
    dsems = [nc.alloc_semaphore(f"d{t}") for t in range(len(tiles))]
    wsem = nc.alloc_semaphore("wsem")
    csem = nc.alloc_semaphore("csem")
    osem = nc.alloc_semaphore("osem")

    # Weights + bias from the scalar queue; the data stream owns sync.
    nc.scalar.dma_start(wp[:, :], wpe[:, :]).then_inc(wsem, 16)
    nc.scalar.dma_start(lbt[:, :], linb[:, :]).then_inc(wsem, 16)
    if nlo:
        nc.scalar.dma_start(sclt[:, :], scl[:, :]).then_inc(wsem, 16)
    for t, (k0, k1) in enumerate(tiles):
        nc.sync.dma_start(
            xt[:, k0 * NB:k1 * NB], xpe[:, k0:k1, :]).then_inc(dsems[t], 16)

    # PE chain: per tile, wait for its (in-order) completion count.  One
    # matmul per chunk: hilo chunks use a [128,2] lhsT (hi and lo weight
    # columns) producing both PSUM rows in one instruction.
    nc.tensor.wait_ge(wsem, 48 if nlo else 32)
    for t, (k0, k1) in enumerate(tiles):
        nc.tensor.wait_ge(dsems[t], 16)
        for k in range(k0, k1):
            rhs = xt[:, k * NB:(k + 1) * NB]
            if k < nlo:
                mm = nc.tensor.matmul(
                    ps[:, :], lhsT=wp[:, 2 * k:2 * k + 2], rhs=rhs,
                    start=(k == 0), stop=False, skip_group_check=True)
            else:
                mm = nc.tensor.matmul(
                    ps[0:1, :], lhsT=wp[:, nlo + k:nlo + k + 1], rhs=rhs,
                    start=(k == 0), stop=(k == NCHUNK - 1),
                    skip_group_check=True)
            if k == NCHUNK - 1:
                mm.then_inc(csem, 1)

    # Combine (ACT folds the lo register + bias, DVE adds the hi register),
    # then the output DMA.  No trailing waits: the NRT postamble fences DMA
    # completion and zeroes the whole semaphore file, so the next execution
    # starts clean.
    if nlo:
        # PSUM row 1 cannot be addressed directly (partition-0 rule), so
        # copy both rows to SBUF and fold them with a 2-deep matmul
        # against (1, 1/LO_SCALE).
        nc.vector.wait_ge(csem, 1)
        nc.vector.tensor_copy(sb2[:, :], ps[:, :]).then_inc(csem, 1)
        nc.tensor.wait_ge(csem, 2)
        nc.tensor.matmul(ps2[:, :], lhsT=sclt[:, :], rhs=sb2[:, :],
                         start=True, stop=True,
                         skip_group_check=True).then_inc(csem, 1)
        nc.vector.wait_ge(csem, 3)
        nc.vector.tensor_scalar(
            res[:, :], ps2[:, :], 1.0 / W_SCALE, lbt[:, :],
            mybir.AluOpType.mult, mybir.AluOpType.add).then_inc(csem, 1)
        nc.sync.wait_ge(csem, 4)
    else:
        nc.vector.wait_ge(csem, 1)
        nc.vector.tensor_scalar(
            res[:, :], ps[0:1, :], 1.0 / W_SCALE, lbt[:, :],
            mybir.AluOpType.mult, mybir.AluOpType.add).then_inc(csem, 1)
        nc.sync.wait_ge(csem, 2)
    nc.sync.dma_start(out[:, :], res[:, :]).then_inc(osem, 16)
    nc.finalize()
    return nc


def _build_bass(cpt=CPT, pebufs=PEBUFS, hilo=HILO):
    nlo = NCROP if hilo else 0
    ntiles = (NCHUNK + cpt - 1) // cpt

    nc = bacc.Bacc("TRN2")
    xpe = nc.dram_tensor("xpe", [128, NCHUNK, NB], _F16, kind="ExternalInput")
    wpe = nc.dram_tensor("wpe", [128, NCHUNK + nlo], _F16,
                         kind="ExternalInput")
    linb = nc.dram_tensor("linb", [1, 1], _F32, kind="ExternalInput")
    out = nc.dram_tensor("out", [1, NB], _F32, kind="ExternalOutput")

    with TileContext(nc) as tc:
        with (
            tc.tile_pool(name="cpool", bufs=1) as cpool,
            tc.tile_pool(name="pepool", bufs=pebufs) as pepool,
            tc.tile_pool(name="apool", bufs=1) as apool,
            tc.tile_pool(name="ppool", bufs=2, space="PSUM") as ppool,
        ):
            # Weights + bias issue from the scalar queue so the data
            # stream owns the sync queue from t=0.
            wp = cpool.tile([128, NCHUNK + nlo], _F16)
            nc.scalar.dma_start(out=wp[:], in_=wpe[:, :])
            lb = cpool.tile([1, 1], _F32)
            nc.scalar.dma_start(out=lb[:], in_=linb[:, :])

            ps_hi = ppool.tile([1, NB], _F32)
            ps_lo = ppool.tile([1, NB], _F32)
            for t in range(ntiles):
                k0 = t * cpt
                k1 = min(k0 + cpt, NCHUNK)
                pt = pepool.tile([128, (k1 - k0) * NB], _F16, tag="pt")
                nc.sync.dma_start(out=pt[:], in_=xpe[:, k0:k1, :])
                for k in range(k0, k1):
                    rhs = pt[:, (k - k0) * NB:(k - k0 + 1) * NB]
                    nc.tensor.matmul(
                        ps_hi[:], lhsT=wp[:, k:k + 1], rhs=rhs,
                        start=(k == 0), stop=(k == NCHUNK - 1))
                    if k < nlo:
                        nc.tensor.matmul(
                            ps_lo[:], lhsT=wp[:, NCHUNK + k:NCHUNK + k + 1],
                            rhs=rhs, start=(k == 0), stop=(k == nlo - 1))

            # Combine the two PSUM registers, undo the weight pre-scales,
            # add lin_b.
            res = apool.tile([1, NB], _F32)
            if nlo:
                half = apool.tile([1, NB], _F32)
                nc.vector.tensor_scalar(
                    half[:], ps_lo[:], 1.0 / (W_SCALE * LO_SCALE), lb[:],
                    mybir.AluOpType.mult, mybir.AluOpType.add)
                nc.vector.scalar_tensor_tensor(
                    out=res[:], in0=ps_hi[:], scalar=1.0 / W_SCALE,
                    in1=half[:],
                    op0=mybir.AluOpType.mult, op1=mybir.AluOpType.add)
            else:
                nc.vector.tensor_scalar(
                    res[:], ps_hi[:], 1.0 / W_SCALE, lb[:],
                    mybir.AluOpType.mult, mybir.AluOpType.add)
            nc.sync.dma_start(out=out[:, :], in_=res[:])
    nc.finalize()
    return nc


def _shard_inputs(x1, x2, x3, share_feature, A, Ws3, lin_b, idx_h, idx_w,
                  hilo=HILO):
    """Gather crops, pack per-core PE-layout tensors (fp16)."""
    nlo = NCROP if hilo else 0

    xc1 = _crop(x1, idx_h[0], idx_w[0])              # [64, 1024, 49]
    xc2 = _crop(x2, idx_h[1], idx_w[1])
    xcs = _crop(share_feature, idx_h[3], idx_w[3])
    Aq1 = A[:, 0:7, 0:7].reshape(1024, 49)
    Aq2 = A[:, 7:14, 0:7].reshape(1024, 49)
    Aq4 = A[:, 7:14, 7:14].reshape(1024, 49)
    x3f = np.asarray(x3, np.float32).reshape(NB, 1280 * 784)
    w3f = Ws3.reshape(1280 * 784)

    in_maps = []
    for m in range(NCORES):
        cs = slice(m * 128, (m + 1) * 128)
        e0, e1 = m * 160 * 784, (m + 1) * 160 * 784

        # Per-core flat element stream: crops (ch-major) ++ x3 slice.
        cropx = np.concatenate(
            [xc1[:, cs], xc2[:, cs], xcs[:, cs]], axis=2)    # [64, 128, 147]
        xall = np.concatenate(
            [cropx.reshape(NB, 128 * NCROP), x3f[:, e0:e1]], axis=1)
        # chunks: [64, 1127, 128] -> [128, 1127, 64]
        xpe = np.ascontiguousarray(
            xall.reshape(NB, NCHUNK, 128).transpose(2, 1, 0),
            dtype=np.float16)

        cropw = np.concatenate(
            [Aq1[cs], Aq2[cs], Aq4[cs]], axis=1)             # [128, 147]
        wall = np.concatenate(
            [cropw.reshape(128 * NCROP), w3f[e0:e1]]) * W_SCALE
        whi = wall.reshape(NCHUNK, 128).T.astype(np.float16)  # [128, 1127]
        if nlo:
            # Columns 0..2*nlo-1 interleave (hi_k, lo_k) pairs for the crop
            # chunks (one [128,2] lhsT per chunk); then the plain hi
            # columns for the rest.
            wlo = ((wall[:128 * NCROP]
                    - whi.T.reshape(NCHUNK * 128)[:128 * NCROP]
                        .astype(np.float64))
                   * LO_SCALE).reshape(nlo, 128).T.astype(np.float16)
            wpe = np.empty((128, NCHUNK + nlo), np.float16)
            wpe[:, 0:2 * nlo:2] = whi[:, :nlo]
            wpe[:, 1:2 * nlo:2] = wlo
            wpe[:, 2 * nlo:] = whi[:, nlo:]
            wpe = np.ascontiguousarray(wpe)
        else:
            wpe = np.ascontiguousarray(whi, dtype=np.float16)

        linb = np.array([[lin_b[0] if m == 0 else 0.0]], np.float32)
        im = {'xpe': xpe, 'wpe': wpe, 'linb': linb}
        if nlo:
            im['scl'] = np.array([[1.0], [1.0 / LO_SCALE]], np.float32)
        in_maps.append(im)
    return in_maps


def _prepare(inputs):
    """Fold weights + shard; returns (nc, in_maps)."""
    A, Ws3 = _build_fold(
        np.asarray(inputs['c_w']), np.asarray(inputs['conv3d_w']),
        np.asarray(inputs['lin_w']), np.asarray(inputs['lin_b']),
        np.asarray(inputs['idx_h']), np.asarray(inputs['idx_w']))
    in_maps = _shard_inputs(
        np.asarray(inputs['x1']), np.asarray(inputs['x2']),
        np.asarray(inputs['x3']), np.asarray(inputs['share_feature']),
        A, Ws3, np.asarray(inputs['lin_b']),
        np.asarray(inputs['idx_h']), np.asarray(inputs['idx_w']))
    nc = _build_bass_raw()
    return nc, in_maps


def _ensure_ntff_hook():
    """Make `trace=True` (e.g. BASS_TRACE=1) work under axon even when the
    image's antenv package lacks axon_hooks: register an equivalent module
    backed by the ctypes NTFF hook from trn_agent_boot."""
    import sys
    import types
    try:
        import antenv.axon_hooks  # noqa: F401
        return
    except Exception:
        pass
    try:
        from trn_agent_boot import trn_boot
        hook = trn_boot._ntff_profile_via_ctypes('/opt/axon/libaxon_pjrt.so')
        mod = types.ModuleType('antenv.axon_hooks')
        mod.get_axon_ntff_profile_hook = lambda: hook
        mod.set_axon_ntff_profile_hook = lambda h: None
        sys.modules['antenv.axon_hooks'] = mod
    except Exception:
        pass


def kernel(x1, x2, x3, share_feature, c_w, conv3d_w, lin_w, lin_b,
           idx_h, idx_w):
    _ensure_ntff_hook()
    nc, in_maps = _prepare({
        'x1': x1, 'x2': x2, 'x3': x3, 'share_feature': share_feature,
        'c_w': c_w, 'conv3d_w': conv3d_w, 'lin_w': lin_w, 'lin_b': lin_b,
        'idx_h': idx_h, 'idx_w': idx_w})
    res = run_bass_kernel_spmd(nc, in_maps, core_ids=list(range(NCORES)))
    parts = np.stack([r['out'][0] for r in res.results])      # [8, 64]
    return parts.sum(axis=0, dtype=np.float64).astype(np.float32).reshape(
        NB, 1)
